# revision 12
# baseline (speedup 1.0000x reference)
"""Trainium2 Bass kernel for nn_AttentionBlock (GroupNorm + 1x1-conv QKV
self-attention + 1x1-conv out-proj + residual).

Full input shapes: x (8, 256, 64, 64) f32, gn_weight/gn_bias (256,),
qkv_w (768, 256), qkv_b (768,), out_w (256, 256), out_b (256,).

Sharding: data-parallel over batch - one batch item per NeuronCore (8 cores).

fp8 DoubleRow design (v3):
  - x is quantized to fp8 (x8) chunk-by-chunk as the DMA lands (ACT), while
    bn_stats chases on DVE. The GroupNorm affine xn = a*x + b is folded into
    the conv weights on device: W' = (W . a) * 4 in fp8 (one tensor_scalar
    per channel chunk), so there is no GN-apply pass at all. The b-offset
    terms become per-channel biases: the k one is dropped (softmax shift
    invariance), the q one is computed by tiny N=1 matmuls, and the v one
    is folded into the out-proj bias on device (softmax rows sum to 1).
  - All 1x1 convs and both attention matmuls run as fp8e4 DoubleRow (K=256
    per instruction). Weights are scaled x4 on host so they sit in e4m3's
    normal range; compensation: exp(scale=1/256) for q.k, and 1/16 folded
    into the softmax-reciprocal broadcast for v/attn.
  - exp runs on ACT from 2-bank PSUM score groups (double-buffered), bias
    -ln(16) keeps es = exp(s)/16 within fp8e4 max (240); the scale cancels
    in the softmax ratio. exp writes fp8 es directly.
  - The softmax denominator is a DoubleRow matmul with an all-ones lhsT
    (every output partition holds the sum; row 0 used) - no DVE add chains.
  - Residual comes from the staged x in SBUF; out-proj bias + residual fuse
    into one scalar_tensor_tensor on DVE.
  - ACT table sets: Sqrt (GroupNorm, once) and Exp; both are front-loaded
    with dummy ops so the ~1.3us loads hide under the DMA/conv phases.
"""

import ml_dtypes
import numpy as np

import concourse.bass as bass
import concourse.tile as tile
from concourse import bacc, mybir
from concourse.bass_utils import run_bass_kernel_spmd

F32 = mybir.dt.float32
F32R = mybir.dt.float32r
BF16 = mybir.dt.bfloat16
FP8 = mybir.dt.float8e4
AF = mybir.ActivationFunctionType
OP = mybir.AluOpType
DR = mybir.MatmulPerfMode.DoubleRow

B = 8          # batch (= cores)
C = 256        # channels
P = 128        # partitions
NCC = C // P   # channel chunks (2)
G = 32         # groups
GS = C // G    # channels per group (8)
GPC = P // GS  # groups per partition chunk (16)
EPS = 1e-5
LN16 = float(np.log(16.0))


def build(hw=4096, iblk=512):
    """Build the per-core Bass program. hw = pixels per image (4096 full)."""
    assert hw % 512 == 0 and hw % iblk == 0 and iblk == 512
    njt = hw // P      # j tiles of 128 (32 full size)
    nib = hw // iblk   # i blocks (8 full size)
    njb = hw // 512    # 512-wide pixel chunks
    neg = njt // 2     # exp groups per block (2 j-tiles each)

    nc = bacc.Bacc("TRN2", target_bir_lowering=False, debug=False, num_devices=B)

    x_d = nc.dram_tensor("x", [NCC, P, hw], F32, kind="ExternalInput").ap()
    qkv_wt_d = nc.dram_tensor(
        "qkv_wt", [NCC, P, 3 * C], BF16, kind="ExternalInput"
    ).ap()
    out_wt_d = nc.dram_tensor(
        "out_wt", [NCC, P, C], FP8, kind="ExternalInput"
    ).ap()
    qb4_d = nc.dram_tensor("qb4", [P, NCC], F32, kind="ExternalInput").ap()
    obias_d = nc.dram_tensor("obias", [P, NCC], F32, kind="ExternalInput").ap()
    gn_w_d = nc.dram_tensor("gn_w", [P, NCC], F32, kind="ExternalInput").ap()
    gn_b_d = nc.dram_tensor("gn_b", [P, NCC], F32, kind="ExternalInput").ap()
    gmask_d = nc.dram_tensor("gmask", [P, GPC], F32, kind="ExternalInput").ap()
    gmaskT_d = nc.dram_tensor("gmaskT", [GPC, P], F32, kind="ExternalInput").ap()
    y_d = nc.dram_tensor("y", [NCC, P, hw], F32, kind="ExternalOutput").ap()

    with tile.TileContext(nc) as tc:
        with (
            tc.tile_pool(name="const", bufs=1) as cst,
            tc.tile_pool(name="xs", bufs=1) as xsp,
            tc.tile_pool(name="x8p", bufs=1) as x8p,
            tc.tile_pool(name="kt", bufs=1) as ktp,
            tc.tile_pool(name="v", bufs=1) as vp,
            tc.tile_pool(name="es", bufs=2) as esp,
            tc.tile_pool(name="work", bufs=2) as wp,
            tc.tile_pool(name="stat", bufs=2) as sp,
            tc.tile_pool(name="ps_s", bufs=2, space="PSUM") as ps_s,
            tc.tile_pool(name="ps_pv", bufs=1, space="PSUM") as ps_pv,
            tc.tile_pool(name="ps_dn", bufs=1, space="PSUM") as ps_dn,
            tc.tile_pool(name="ps_m", bufs=1, space="PSUM") as ps_m,
        ):
            # ---- x DMA first (cc-interleaved chunks), weights after ----
            xs = xsp.tile([P, NCC, hw], F32)      # staged x (also residual)
            x8 = x8p.tile([P, NCC, hw], FP8)      # fp8 copy for the convs
            for h4 in range(njb // 2):
                nc.sync.dma_start(
                    out=xs[:, 0, h4 * 1024:(h4 + 1) * 1024],
                    in_=x_d[0, :, h4 * 1024:(h4 + 1) * 1024],
                )
                nc.scalar.dma_start(
                    out=xs[:, 1, h4 * 1024:(h4 + 1) * 1024],
                    in_=x_d[1, :, h4 * 1024:(h4 + 1) * 1024],
                )

            qkv_wt = cst.tile([P, NCC, 3 * C], BF16)
            out_wt = cst.tile([P, NCC, C], FP8)
            qb4 = cst.tile([P, NCC], F32)
            obias_h = cst.tile([P, NCC], F32)
            gn_w = cst.tile([P, NCC], F32)
            gn_b = cst.tile([P, NCC], F32)
            gmask = cst.tile([P, GPC], F32)
            gmaskT = cst.tile([GPC, P], F32)
            # DR denominator lhsT; 16.0 folds the out_w*4 / v*4 comp into
            # the reciprocal (rd = 1/(16*den) = 0.0625/den)
            ones8 = cst.tile([P, 2, P], FP8)
            eps_t = cst.tile([GPC, 1], F32)
            nln16 = cst.tile([P, 1], F32)
            for cc in range(NCC):
                nc.sync.dma_start(out=qkv_wt[:, cc, :], in_=qkv_wt_d[cc])
                nc.sync.dma_start(out=out_wt[:, cc, :], in_=out_wt_d[cc])
            nc.sync.dma_start(out=qb4, in_=qb4_d[:, :])
            nc.sync.dma_start(out=obias_h, in_=obias_d[:, :])
            nc.sync.dma_start(out=gn_w, in_=gn_w_d[:, :])
            nc.sync.dma_start(out=gn_b, in_=gn_b_d[:, :])
            nc.sync.dma_start(out=gmask, in_=gmask_d[:, :])
            nc.sync.dma_start(out=gmaskT, in_=gmaskT_d[:, :])
            nc.vector.memset(ones8, 16.0)
            nc.vector.memset(eps_t, EPS)
            nc.vector.memset(nln16, -LN16)

            # front-load the exp table set (the only one the kernel uses)
            dmy = sp.tile([P, 1], F32, tag="dmy")
            nc.vector.memset(dmy, 1.0)
            nc.scalar.activation(dmy, dmy, AF.Exp)

            # PE warm-up during the DMA head (keeps HAM at full clock)
            wrm = ps_m.tile([P, P], F32, tag="mm")
            for _ in range(20):
                nc.tensor.matmul(
                    wrm, ones8, ones8[:, :, 0:P], start=True,
                    stop=True, perf_mode=DR, skip_group_check=True,
                )
            wrs = sp.tile([P, 1], F32, tag="wrs")
            nc.vector.tensor_copy(wrs, wrm[:, 0:1])

            # chase the DMA: bn_stats (DVE) per 512; fp8 cast (ACT) per 1024
            stats = sp.tile([P, NCC, njb, 6], F32, tag="bnst")
            for h2 in range(njb):
                for cc in range(NCC):
                    sl = slice(h2 * 512, (h2 + 1) * 512)
                    nc.vector.bn_stats(out=stats[:, cc, h2, :], in_=xs[:, cc, sl])
                if h2 % 2 == 1:
                    for cc in range(NCC):
                        sl2 = slice((h2 - 1) * 512, (h2 + 1) * 512)
                        nc.scalar.activation(x8[:, cc, sl2], xs[:, cc, sl2], AF.Copy)

            # persistent attention tensors
            kt8 = ktp.tile([P, NCC, hw], FP8)     # k in (c, j) layout
            v8 = vp.tile([P, njt, C], FP8)        # v in (j, c) layout

            # ---- GroupNorm stats -> per-row scale a_t / offset b_t ----
            # batched over both channel chunks; rsqrt via bit-trick + 2
            # Newton steps on DVE (no Sqrt table set needed)
            ab = sp.tile([P, NCC, 2], F32, tag="ab")
            tt = sp.tile([P, 2, 2], F32, tag="t2")  # [:, cc, {mean, E[x^2]}]
            for cc in range(NCC):
                mv = sp.tile([P, 2], F32, tag="mv", name=f"mv{cc}")
                nc.vector.bn_aggr(out=mv, in_=stats[:, cc, :, :])
                nc.vector.tensor_copy(tt[:, cc, 0:1], mv[:, 0:1])
                nc.vector.tensor_mul(tt[:, cc, 1:2], mv[:, 0:1], mv[:, 0:1])
                nc.vector.tensor_add(tt[:, cc, 1:2], tt[:, cc, 1:2], mv[:, 1:2])
            gsum = ps_m.tile([GPC, 4], F32, tag="mm")
            nc.tensor.matmul(gsum, gmask, tt, start=True, stop=True)
            gstat = sp.tile([GPC, 2, 2], F32, tag="gstat")
            nc.vector.tensor_scalar(
                out=gstat, in0=gsum, scalar1=1.0 / GS, scalar2=None, op0=OP.mult
            )
            gm = gstat[:, :, 0:1]                  # means  [GPC, 2, 1]
            z = sp.tile([GPC, 2], F32, tag="gvar")  # var + eps
            nc.vector.tensor_mul(z, gm[:, :, 0], gm[:, :, 0])
            nc.vector.tensor_sub(z, gstat[:, :, 1], z)
            nc.vector.tensor_scalar(
                out=z, in0=z, scalar1=float(EPS), scalar2=None, op0=OP.add
            )
            # rsqrt(z): y0 = bits(0x5f3759df - (z_bits >> 1)); 2 Newton steps
            magic = sp.tile([GPC, 2], mybir.dt.int32, tag="magic")
            nc.vector.memset(magic, 0x5F3759DF)
            ybits = sp.tile([GPC, 2], mybir.dt.int32, tag="ybits")
            nc.vector.tensor_scalar(
                out=ybits, in0=z.bitcast(mybir.dt.int32), scalar1=1,
                scalar2=None, op0=OP.logical_shift_right,
            )
            nc.vector.tensor_sub(ybits, magic, ybits)
            y = ybits.bitcast(F32)
            h = sp.tile([GPC, 2], F32, tag="hh")
            nc.vector.tensor_scalar(
                out=h, in0=z, scalar1=0.5, scalar2=None, op0=OP.mult
            )
            t1 = sp.tile([GPC, 2], F32, tag="t1")
            for _ in range(2):
                nc.vector.tensor_mul(t1, y, y)
                nc.vector.tensor_mul(t1, t1, h)
                nc.vector.tensor_scalar(
                    out=t1, in0=t1, scalar1=-1.0, scalar2=1.5,
                    op0=OP.mult, op1=OP.add,
                )
                nc.vector.tensor_mul(y, y, t1)
            gmr = sp.tile([GPC, 2, 2], F32, tag="gmr")  # {mean, rstd} per cc
            nc.vector.tensor_copy(gmr[:, :, 0], gm[:, :, 0])
            nc.vector.tensor_copy(gmr[:, :, 1], y)
            bcp = ps_m.tile([P, 4], F32, tag="mm")
            nc.tensor.matmul(bcp, gmaskT, gmr, start=True, stop=True)
            rowst = sp.tile([P, 2, 2], F32, tag="rowst")
            nc.vector.tensor_copy(rowst, bcp)
            for cc in range(NCC):
                # a = rstd*w ; b = gn_b - mean*a
                nc.vector.tensor_mul(
                    ab[:, cc, 0:1], rowst[:, cc, 1:2], gn_w[:, cc:cc + 1]
                )
                nc.vector.tensor_mul(ab[:, cc, 1:2], rowst[:, cc, 0:1], ab[:, cc, 0:1])
                nc.vector.tensor_sub(
                    ab[:, cc, 1:2], gn_b[:, cc:cc + 1], ab[:, cc, 1:2]
                )

            # ---- fold GN scale into fp8 conv weights: W8 = (W . a) * 4 ----
            qkv_w8 = cst.tile([P, NCC, 3 * C], FP8)
            a4 = sp.tile([P, NCC], F32, tag="a4")
            for cc in range(NCC):
                nc.vector.tensor_scalar(
                    out=a4[:, cc:cc + 1], in0=ab[:, cc, 0:1], scalar1=4.0,
                    scalar2=None, op0=OP.mult,
                )
            nc.vector.tensor_scalar(
                out=qkv_w8[:, 0, :], in0=qkv_wt[:, 0, :],
                scalar1=a4[:, 0:1], scalar2=None, op0=OP.mult,
            )
            nc.scalar.activation(
                qkv_w8[:, 1, :], qkv_wt[:, 1, :], AF.Identity,
                bias=0.0, scale=a4[:, 1:2],
            )

            # ---- GN-offset bias terms (tiny N=1 matmuls) ----
            b16 = sp.tile([P, NCC], BF16, tag="b16")
            for cc in range(NCC):
                nc.vector.tensor_copy(b16[:, cc:cc + 1], ab[:, cc, 1:2])
            # q4 = W8q @ x8 + qbias where qbias = 4*(Wq @ b) + 4*qb
            qbias = sp.tile([P, NCC], F32, tag="qbias")
            vbias8 = sp.tile([P, NCC], FP8, tag="vbias8")
            for oc in range(NCC):
                pqb = ps_m.tile([P, 1], F32, tag="mm", name=f"pqb{oc}")
                for cc in range(NCC):
                    nc.tensor.matmul(
                        pqb,
                        qkv_wt[:, cc, oc * P:(oc + 1) * P],
                        b16[:, cc:cc + 1],
                        start=(cc == 0), stop=(cc == NCC - 1),
                    )
                nc.vector.scalar_tensor_tensor(
                    out=qbias[:, oc:oc + 1], in0=pqb, scalar=4.0,
                    in1=qb4[:, oc:oc + 1], op0=OP.mult, op1=OP.add,
                )
            # vb_eff = Wv @ b (raw weights); obias += out_w @ vb_eff
            for oc in range(NCC):
                pvb = ps_m.tile([P, 1], F32, tag="mm", name=f"pvb{oc}")
                for cc in range(NCC):
                    nc.tensor.matmul(
                        pvb,
                        qkv_wt[:, cc, 2 * C + oc * P:2 * C + (oc + 1) * P],
                        b16[:, cc:cc + 1],
                        start=(cc == 0), stop=(cc == NCC - 1),
                    )
                nc.vector.tensor_copy(vbias8[:, oc:oc + 1], pvb)
            obias = sp.tile([P, NCC], F32, tag="obias_d")
            for o2 in range(NCC):
                pob = ps_m.tile([P, 1], F32, tag="mm", name=f"pob{o2}")
                for cc in range(NCC):
                    nc.tensor.matmul(
                        pob,
                        out_wt[:, cc, o2 * P:(o2 + 1) * P],
                        vbias8[:, cc:cc + 1],
                        start=(cc == 0), stop=(cc == NCC - 1),
                    )
                # out_wt is 4*out_w -> scale by 1/4
                nc.vector.scalar_tensor_tensor(
                    out=obias[:, o2:o2 + 1], in0=pob, scalar=0.25,
                    in1=obias_h[:, o2:o2 + 1], op0=OP.mult, op1=OP.add,
                )

            # ---- attention block machinery ----
            st = {}

            def emit_qt(ib):
                isl = slice(ib * iblk, (ib + 1) * iblk)
                qt8 = wp.tile([P, NCC, iblk], FP8, tag="qt", name=f"qt{ib}")
                for oc in range(NCC):
                    pq = ps_m.tile([P, iblk], F32, tag="mm", name=f"pq{ib}_{oc}")
                    nc.tensor.matmul(
                        pq,
                        qkv_w8[:, :, oc * P:(oc + 1) * P],
                        x8[:, :, isl],
                        start=True, stop=True, perf_mode=DR,
                    )
                    nc.vector.tensor_scalar(
                        out=qt8[:, oc, :], in0=pq, scalar1=qbias[:, oc:oc + 1],
                        scalar2=None, op0=OP.add,
                    )
                st.setdefault(ib, {})["qt"] = qt8

            def alloc_block(ib):
                st.setdefault(ib, {})
                st[ib]["es"] = esp.tile(
                    [P, njt, iblk], FP8, tag="es", name=f"es{ib}"
                )
                st[ib]["pv"] = ps_pv.tile(
                    [P, NCC, iblk], F32, tag="pv", name=f"pv{ib}"
                )
                st[ib]["dn"] = ps_dn.tile(
                    [P, iblk], F32, tag="dn", name=f"dn{ib}"
                )

            def emit_scores_group(ib, g):
                qt8 = st[ib]["qt"]
                es = st[ib]["es"]
                ps = ps_s.tile([P, 2, iblk], F32, tag="sc", name=f"ps{ib}_{g}")
                for k in range(2):
                    jt = g * 2 + k
                    nc.tensor.matmul(
                        ps[:, k, :],
                        kt8[:, :, jt * P:(jt + 1) * P],
                        qt8,
                        start=True, stop=True,
                        perf_mode=DR,
                    )
                nc.scalar.activation(
                    es[:, g * 2:(g + 1) * 2, :], ps, AF.Exp,
                    bias=nln16, scale=1.0 / 256.0,
                )

            def emit_pv_pair(ib, t):
                es = st[ib]["es"]
                pvp = st[ib]["pv"]
                dn = st[ib]["dn"]
                for oc in range(NCC):
                    nc.tensor.matmul(
                        pvp[:, oc, :],
                        v8[:, 2 * t:2 * t + 2, oc * P:(oc + 1) * P],
                        es[:, 2 * t:2 * t + 2, :],
                        start=(t == 0), stop=(t == njt // 2 - 1),
                        perf_mode=DR,
                        skip_group_check=True,
                    )
                nc.tensor.matmul(
                    dn,
                    ones8,
                    es[:, 2 * t:2 * t + 2, :],
                    start=(t == 0), stop=(t == njt // 2 - 1),
                    perf_mode=DR,
                    skip_group_check=True,
                )

            def emit_denfinish(ib):
                rd = wp.tile([1, iblk], F32, tag="rd", name=f"rd{ib}")
                nc.vector.reciprocal_approx_fast(rd, st[ib]["dn"][0:1, :])
                rb = wp.tile([P, iblk], F32, tag="rb", name=f"rb{ib}")
                nc.gpsimd.partition_broadcast(rb, rd)
                st[ib]["rb"] = rb

            def emit_normalize(ib):
                attn8 = wp.tile([P, NCC, iblk], FP8, tag="attn", name=f"at{ib}")
                for oc in range(NCC):
                    nc.vector.tensor_mul(
                        attn8[:, oc, :], st[ib]["pv"][:, oc, :], st[ib]["rb"]
                    )
                st[ib]["attn"] = attn8

            def emit_outproj(ib, o2):
                isl = slice(ib * iblk, (ib + 1) * iblk)
                py = ps_m.tile([P, iblk], F32, tag="mm", name=f"py{ib}_{o2}")
                nc.tensor.matmul(
                    py,
                    out_wt[:, :, o2 * P:(o2 + 1) * P],
                    st[ib]["attn"],
                    start=True, stop=True,
                    perf_mode=DR,
                )
                yo = wp.tile([P, iblk], F32, tag="yo", bufs=4, name=f"yo{ib}_{o2}")
                nc.vector.scalar_tensor_tensor(
                    out=yo, in0=py, scalar=obias[:, o2:o2 + 1],
                    in1=xs[:, o2, isl], op0=OP.add, op1=OP.add,
                )
                nc.sync.dma_start(out=y_d[o2, :, isl], in_=yo)
                if o2 == NCC - 1:
                    del st[ib]

            # ---- conv phase (kconv drains on ACT, vconv on DVE) ----
            emit_qt(0)
            for jb in range(njb):
                pk = ps_s.tile([P, NCC, 512], F32, tag="sc", name=f"pk{jb}")
                for oc in range(NCC):
                    nc.tensor.matmul(
                        pk[:, oc, :],
                        qkv_w8[:, :, C + oc * P:C + (oc + 1) * P],
                        x8[:, :, jb * 512:(jb + 1) * 512],
                        start=True, stop=True, perf_mode=DR,
                    )
                nc.scalar.activation(
                    kt8[:, :, jb * 512:(jb + 1) * 512], pk, AF.Copy
                )
                pv = ps_pv.tile([P, 2, 512], F32, tag="pv", name=f"pvc{jb}")
                for k in range(4):
                    jt = jb * 4 + k
                    nc.tensor.matmul(
                        pv[:, k // 2, (k % 2) * C:(k % 2 + 1) * C],
                        x8[:, :, jt * P:(jt + 1) * P],
                        qkv_w8[:, :, 2 * C:3 * C],
                        start=True, stop=True, perf_mode=DR,
                        skip_group_check=True,
                    )
                nc.vector.tensor_copy(v8[:, jb * 4:(jb + 1) * 4, :], pv)

            # ---- blocks 0..7 steady state ----
            # per block: scores g0/g1 interleave with the previous block's
            # spill pv pairs (14, 15); denfinish after pv15; own pv pairs
            # lag 4 groups; pairs 12, 13 after the loop; 14, 15 spill.
            for ib in range(nib):
                last = ib == nib - 1
                alloc_block(ib)
                for g in range(neg):
                    emit_scores_group(ib, g)
                    if ib > 0:
                        if g == 0:
                            emit_pv_pair(ib - 1, njt // 2 - 2)
                        elif g == 1:
                            emit_pv_pair(ib - 1, njt // 2 - 1)
                            emit_denfinish(ib - 1)
                        elif g == 2:
                            emit_normalize(ib - 1)
                        elif g == 5:
                            emit_outproj(ib - 1, 0)
                        elif g == 6:
                            emit_outproj(ib - 1, 1)
                    if g == 7 and ib < nib - 1:
                        emit_qt(ib + 1)
                    if not last:
                        if g >= 4:
                            emit_pv_pair(ib, g - 4)
                    else:
                        # final block: chase tighter so the tail chain
                        # starts as soon as the last exp lands
                        if g >= 2:
                            emit_pv_pair(ib, g - 2)
                if not last:
                    emit_pv_pair(ib, neg - 4)
                    emit_pv_pair(ib, neg - 3)
            emit_pv_pair(nib - 1, njt // 2 - 2)
            emit_pv_pair(nib - 1, njt // 2 - 1)
            emit_denfinish(nib - 1)
            emit_normalize(nib - 1)
            emit_outproj(nib - 1, 0)
            emit_outproj(nib - 1, 1)

    nc.compile()
    return nc


def prep_inputs(x, gn_weight, gn_bias, qkv_w, qkv_b, out_w, out_b, hw=4096):
    """Host-side layout prep. Returns per-core input maps."""
    b = x.shape[0]
    e4 = ml_dtypes.float8_e4m3
    # raw qkv weights in bf16; the device folds in 4*a (GN scale + e4m3
    # range), compensated by exp scale 1/256 for q.k and 1/16 in the
    # reciprocal broadcast for v/attn.
    qkv_wt = np.ascontiguousarray(
        qkv_w.astype(np.float32).T.reshape(NCC, P, 3 * C)
    ).astype(ml_dtypes.bfloat16)
    out_wt = np.ascontiguousarray(
        (out_w * 4.0).T.reshape(NCC, P, C)
    ).astype(e4)
    qb4 = np.ascontiguousarray(
        (qkv_b[:C] * 4.0).reshape(NCC, P).T
    ).astype(np.float32)
    vb = qkv_b[2 * C:]
    ob = out_b + out_w @ vb
    obias = np.ascontiguousarray(ob.reshape(NCC, P).T).astype(np.float32)
    gn_w2 = np.ascontiguousarray(gn_weight.reshape(NCC, P).T).astype(np.float32)
    gn_b2 = np.ascontiguousarray(gn_bias.reshape(NCC, P).T).astype(np.float32)
    gmask = np.zeros((P, GPC), np.float32)
    gmask[np.arange(P), np.arange(P) // GS] = 1.0
    gmaskT = np.ascontiguousarray(gmask.T)

    shared = dict(
        qkv_wt=qkv_wt, out_wt=out_wt, qb4=qb4, obias=obias,
        gn_w=gn_w2, gn_b=gn_b2, gmask=gmask, gmaskT=gmaskT,
    )
    in_maps = []
    for i in range(b):
        m = dict(shared)
        m["x"] = np.ascontiguousarray(
            x[i].reshape(C, hw).reshape(NCC, P, hw)
        ).astype(np.float32)
        in_maps.append(m)
    return in_maps


_NC_CACHE = {}


def get_nc(hw=4096, iblk=512):
    key = (hw, iblk)
    if key not in _NC_CACHE:
        _NC_CACHE[key] = build(hw, iblk)
    return _NC_CACHE[key]


def kernel(x, gn_weight, gn_bias, qkv_w, qkv_b, out_w, out_b):
    b, c, h, w = x.shape
    assert (b, c) == (B, C)
    hw = h * w
    nc = get_nc(hw=hw)
    in_maps = prep_inputs(x, gn_weight, gn_bias, qkv_w, qkv_b, out_w, out_b, hw=hw)
    res = run_bass_kernel_spmd(nc, in_maps, core_ids=list(range(B)))
    out = np.stack(
        [res.results[i]["y"].reshape(C, h, w) for i in range(b)]
    ).astype(np.float32)
    return out
